# revision 1
# baseline (speedup 1.0000x reference)
"""Trainium2 Bass kernel: 4-layer GCN message passing on 8 NeuronCores.

Self-contained: host-side graph partitioning/padded-CSR planning + Bass/Tile
device program (matmul -> AllGather -> bulk dma_gather -> tree segment-reduce),
executed via run_bass_kernel_spmd on cores 0-7.

kernel(**inputs) takes the FULL unsharded inputs (x, edge_index, lin_w, gcn_w,
gcn_b) and returns the FULL [N, 128] float32 output.
"""


import math
from dataclasses import dataclass, field

import numpy as np
import ml_dtypes

import concourse.bacc as bacc
import concourse.bass as bass
import concourse.mybir as mybir
import concourse.tile as tile
from concourse.bass_utils import run_bass_kernel_spmd
from concourse.masks import make_identity
from concourse import library_config

BF16 = ml_dtypes.bfloat16
SPAN = 8      # groups per uniform-K reduce span
MAXCH = 96    # max msgs chunks (128 slots each) per dma_gather batch


@dataclass
class Plan:
    C: int
    L: int
    N: int
    G: int        # real groups of 128 pi-slots per core
    P: int        # pi-slots per core = G*128
    Nc: int       # real nodes per core
    BG: int       # bounce groups (incl zero group(s))
    BR: int       # bounce rows = BG*128
    HR: int       # rows per window half
    WSZ: int      # table rows per window = C*HR
    COLS: int     # idx stream columns (16 idx each)
    batches: list = field(default_factory=list)
    # host-side tensors (per core where applicable)
    inputs: dict = field(default_factory=dict)
    core_of: np.ndarray = None
    pos_of: np.ndarray = None


def build_plan(x, edge_index, lin_w, gcn_w, gcn_b, C=8):
    N, D = x.shape
    L = lin_w.shape[0]
    assert D == 128 and N % C == 0
    x = np.asarray(x, np.float32)
    src = np.asarray(edge_index[0], np.int64)
    dst = np.asarray(edge_index[1], np.int64)

    deg = (np.bincount(dst, minlength=N) + 1).astype(np.float64)
    dinv = 1.0 / np.sqrt(deg)

    # degree-balanced round-robin node->core assignment
    order = np.argsort(-deg, kind="stable")
    core_of = np.empty(N, np.int64)
    core_of[order] = np.arange(N) % C

    Nc = N // C
    G = (Nc + 127) // 128
    P = G * 128
    BG = G + 1
    if BG % 2:
        BG += 1
    BR = BG * 128
    HR = BR // 2
    while max(HR, 128 + Nc) >= BR:
        BG += 2
        BR = BG * 128
        HR = BR // 2
    WSZ = C * HR
    assert WSZ <= 32768, f"window {WSZ} exceeds int16 index range"
    zrow_w1 = max(HR, 128 + Nc)   # first all-zero bounce row in window 1

    # Greedy per-node window choice: place each src so its dsts' in-edge
    # counts stay balanced across the two windows (cuts padded-CSR K).
    # Global capacities; cores are re-dealt afterwards.
    cap0_full = min(HR - 128, Nc)         # pos < cap0 -> window 0
    gcap = np.array([C * cap0_full, C * (Nc - cap0_full)], np.int64)
    asrc = src
    adst = dst
    aso = np.argsort(asrc, kind="stable")
    adj_dst = adst[aso]
    a_start = np.searchsorted(asrc[aso], np.arange(N + 1))
    out_deg = np.diff(a_start)
    sorder = np.argsort(-out_deg, kind="stable")
    cnt01 = np.zeros((2, N), np.int32)
    win_node = np.empty(N, np.int8)
    for n in sorder:
        if gcap[0] <= 0:
            w = 1
        elif gcap[1] <= 0:
            w = 0
        else:
            dl = adj_dst[a_start[n] : a_start[n + 1]]
            s0 = int(cnt01[0, dl].sum())
            s1 = int(cnt01[1, dl].sum())
            if s0 != s1:
                w = 0 if s0 < s1 else 1
            else:
                w = 0 if gcap[0] >= gcap[1] else 1
        win_node[n] = w
        gcap[w] -= 1
        np.add.at(cnt01[w], adj_dst[a_start[n] : a_start[n + 1]], 1)

    # per-node in-edge window counts (K0, K1) incl self loop
    k0 = np.bincount(dst[win_node[src] == 0], minlength=N)
    k1 = np.bincount(dst[win_node[src] == 1], minlength=N)

    # re-deal: within each window class, sort by (K0+K1, K0) desc and deal
    # round-robin so every core's group g has a near-identical K profile
    core_of = np.empty(N, np.int64)
    pos_of = np.empty(N, np.int64)
    for w in (0, 1):
        base = 0 if w == 0 else cap0_full
        members = np.where(win_node == w)[0]
        key = (k0[members] + k1[members]) * 1000 + np.maximum(k0[members],
                                                              k1[members])
        members = members[np.argsort(-key, kind="stable")]
        core_of[members] = np.arange(len(members)) % C
        pos_of[members] = base + np.arange(len(members)) // C

    rows = 128 + pos_of
    win_of_node = rows // HR                      # 0 or 1
    rel_of_node = core_of * HR + (rows % HR)      # window-relative table slot
    assert (win_of_node == win_node).all()

    # edges keyed by (dst core, dst pos, src window). Self-loops are NOT
    # gathered: the device initializes agg = dinv^2-scaled local h instead.
    es = src
    ed = dst
    kc = core_of[ed]
    kp = pos_of[ed]
    kw = win_of_node[es]
    key = (kc * P + kp) * 2 + kw
    so = np.argsort(key, kind="stable")
    key_s = key[so]
    val_s = rel_of_node[es[so]]
    cnt = np.bincount(key_s, minlength=C * P * 2)
    starts = np.concatenate([[0], np.cumsum(cnt)])[:-1]
    rank_in = np.arange(len(key_s)) - starts[key_s]

    # per-group per-window K: cross-core max (groups are K-profile matched
    # across cores by the re-deal, so this is tight)
    Kg = cnt.reshape(C, G, 128, 2).max(axis=(0, 2))   # [G, 2]
    Kmax = max(int(Kg.max()), 1)

    z0 = 0                   # row 0 of rank 0 block (zero group)
    z1 = zrow_w1 - HR        # rank 0 block, window 1

    idxtab = np.empty((C * P * 2, Kmax), np.int16)
    idxtab.reshape(C * P, 2, Kmax)[:, 0, :] = z0
    idxtab.reshape(C * P, 2, Kmax)[:, 1, :] = z1
    assert val_s.max() < 32768
    assert rank_in.max() < Kmax or Kmax >= cnt.max()
    idxtab[key_s, rank_in] = val_s.astype(np.int16)

    # Rectangles: decompose each window's (group, t) active staircase into
    # maximal (g0, m) x (t0, T) rectangles. Each rectangle is reduced with an
    # in-place binary tree over its T chunk-slabs, then one add into agg.
    def extract_rects(Kw, g0, m, t0):
        if m == 0:
            return []
        kmin = int(Kw[g0 : g0 + m].min())
        out = []
        if kmin > t0:
            out.append((g0, m, t0, kmin - t0))
        # split at groups with Kg == kmin, recurse on sub-runs above kmin
        g = g0
        while g < g0 + m:
            if Kw[g] == kmin:
                g += 1
                continue
            g1 = g
            while g1 < g0 + m and Kw[g1] > kmin:
                g1 += 1
            out.extend(extract_rects(Kw, g, g1 - g, kmin))
            g = g1
        return out

    rects = []
    for w in (0, 1):
        Kw = Kg[:, w]
        g = 0
        while g < G:
            if Kw[g] == 0:
                g += 1
                continue
            g1 = g
            while g1 < G and Kw[g1] > 0:
                g1 += 1
            for (rg0, rm, rt0, rT) in extract_rects(Kw, g, g1 - g, 0):
                rects.append((w, rg0, rm, rt0, rT))
            g = g1

    # split rectangles that exceed MAXCH chunks (along t), then pack into
    # batches (single window per batch, <= MAXCH chunks)
    split = []
    for (w, g0, m, t0, T) in rects:
        assert m <= MAXCH, f"group run m={m} exceeds MAXCH={MAXCH}"
        tcap = max(1, MAXCH // m)
        t = t0
        while t < t0 + T:
            tt = min(tcap, t0 + T - t)
            split.append((w, g0, m, t, tt))
            t += tt
    batches = []
    cur = None
    for (w, g0, m, t0, T) in split:
        if cur is None or cur["w"] != w or cur["chunks"] + m * T > MAXCH:
            cur = {"w": w, "chunks": 0, "rects": []}
            batches.append(cur)
        cur["rects"].append((g0, m, t0, T, cur["chunks"]))
        cur["chunks"] += m * T
    off = 0
    for b in batches:
        b["n"] = b["chunks"] * 128
        b["col"] = off
        off += b["n"] // 16
    COLS = off

    # per-core idx streams: per rect, [T, m, 128] slab (t-major chunks)
    tabv = idxtab.reshape(C, P, 2, Kmax)
    streams = np.empty((C, 16, COLS), np.int16)
    for c in range(C):
        parts = []
        for b in batches:
            for (g0, m, t0, T, _) in b["rects"]:
                slab = tabv[c, g0 * 128 : (g0 + m) * 128, b["w"], t0 : t0 + T]
                parts.append(np.ascontiguousarray(slab.T).ravel())  # [T, m*128]
        s = np.concatenate(parts)
        streams[c] = s.reshape(-1, 16).T
    idxs_in = np.tile(streams, (1, 8, 1))   # replicate over 8 q7 cores -> [C,128,COLS]

    # dense per-core inputs
    nid = np.full((C, P), 0, np.int64)
    vmask = np.zeros((C, P), bool)
    nid[core_of, pos_of] = np.arange(N)
    vmask[core_of, pos_of] = True
    dinv_loc = np.where(vmask, dinv[nid], 0.0)                       # [C,P] f64
    xloc = np.where(vmask[..., None], x[nid], 0.0) * dinv_loc[..., None]
    xt_in = np.ascontiguousarray(xloc.transpose(0, 2, 1)).astype(BF16)  # [C,128,P]
    dinv2_in = np.broadcast_to(
        (dinv_loc**2).astype(np.float32)[:, None, :], (C, 128, P)
    ).copy()
    bias = np.asarray(gcn_b, np.float64)                             # [L,128]
    bd_in = (bias[None, :, :, None] * dinv_loc[:, None, None, :]).astype(BF16)  # [C,L,128,P]
    dinvc_in = np.ascontiguousarray(
        dinv_loc.reshape(C, G, 128).transpose(0, 2, 1)
    ).astype(np.float32)                                             # [C,128,G]
    brep_in = np.broadcast_to(
        bias[L - 1].astype(np.float32)[None, :], (128, 128)
    ).copy()                                                         # [128,128]
    W = np.einsum("lij,ljk->lik", np.asarray(gcn_w, np.float64),
                  np.asarray(lin_w, np.float64))                     # [L,128,128]
    wt_in = np.ascontiguousarray(W.transpose(0, 2, 1)).astype(BF16)  # [L,128,128]

    p = Plan(C=C, L=L, N=N, G=G, P=P, Nc=Nc, BG=BG, BR=BR, HR=HR, WSZ=WSZ,
             COLS=COLS, batches=batches, core_of=core_of, pos_of=pos_of)
    p.inputs = dict(xt=xt_in, dinv2=dinv2_in, bd=bd_in, dinvc=dinvc_in,
                    brep=brep_in, wt=wt_in, idxs=idxs_in)
    return p


def build_nc(p: Plan, debug_mode=""):
    no_gather = "nogather" in debug_mode
    no_reduce = "noreduce" in debug_mode
    no_cc = "nocc" in debug_mode
    C, L, G, P, BG, BR, HR, WSZ, COLS = (
        p.C, p.L, p.G, p.P, p.BG, p.BR, p.HR, p.WSZ, p.COLS)
    DT = mybir.dt
    nc = bacc.Bacc(None, target_bir_lowering=False, num_swdge_queues=4)

    xt_in = nc.dram_tensor("xt", [128, P], DT.bfloat16, kind="ExternalInput")
    dinv2_in = nc.dram_tensor("dinv2", [128, P], DT.float32, kind="ExternalInput")
    bd_in = nc.dram_tensor("bd", [L, 128, P], DT.bfloat16, kind="ExternalInput")
    dinvc_in = nc.dram_tensor("dinvc", [128, G], DT.float32, kind="ExternalInput")
    brep_in = nc.dram_tensor("brep", [128, 128], DT.float32, kind="ExternalInput")
    wt_in = nc.dram_tensor("wt", [L, 128, 128], DT.bfloat16, kind="ExternalInput")
    idx_in = nc.dram_tensor("idxs", [128, COLS], DT.int16, kind="ExternalInput")
    out_ext = nc.dram_tensor("out", [P, 128], DT.float32, kind="ExternalOutput")

    with tile.TileContext(nc) as tc:
        with (
            tc.tile_pool(name="persist", bufs=1) as pp,
            tc.tile_pool(name="dram", bufs=1, space="DRAM") as dp,
            tc.tile_pool(name="msgs", bufs=3) as mp,
            tc.tile_pool(name="work", bufs=4) as wp,
            tc.tile_pool(name="psum", bufs=4, space="PSUM") as psp,
        ):
            bounces = [dp.tile([HR, 128], DT.bfloat16, name=f"bounce{w}")
                       for w in (0, 1)]
            tables = [
                [
                    dp.tile([WSZ, 128], DT.bfloat16, name=f"tab{l}_{w}")
                    for w in (0, 1)
                ]
                for l in range(L)
            ]

            xt_sb = pp.tile([128, P], DT.bfloat16, name="xt_sb")
            agg_sb = pp.tile([128, P], DT.bfloat16, name="agg_sb")
            h_sb = pp.tile([128, BG * 128], DT.bfloat16, name="h_sb")
            dinv2_sb = pp.tile([128, P], DT.float32, name="dinv2_sb")
            bd_sb = pp.tile([128, P], DT.bfloat16, name="bd_sb")
            dinvc_sb = pp.tile([128, G], DT.float32, name="dinvc_sb")
            brep_sb = pp.tile([128, 128], DT.float32, name="brep_sb")
            wt_sb = pp.tile([128, L * 128], DT.bfloat16, name="wt_sb")
            idx_sb = pp.tile([128, COLS], DT.int16, name="idx_sb")
            ident = pp.tile([128, 128], DT.bfloat16, name="ident")

            nc.sync.dma_start(xt_sb[:], xt_in[:])
            nc.sync.dma_start(dinv2_sb[:], dinv2_in[:])
            nc.sync.dma_start(dinvc_sb[:], dinvc_in[:])
            nc.sync.dma_start(brep_sb[:], brep_in[:])
            nc.sync.dma_start(idx_sb[:], idx_in[:])
            for l in range(L):
                nc.sync.dma_start(wt_sb[:, l * 128 : (l + 1) * 128],
                                  wt_in[l, :, :])
            make_identity(nc, ident[:])
            if no_gather or no_reduce:
                nc.vector.memset(agg_sb[:], 0.0)
            # GPSIMD library switches (standard <-> mlp for DMAGatherAnt) are
            # auto-inserted by Bacc.compile's insert_library_loads pass.

            # zero group 0 (+ any trailing zero groups) of h, mirror into bounce
            nc.vector.memset(h_sb[:, :128], 0.0)
            if BG > G + 1:
                nc.vector.memset(h_sb[:, (G + 1) * 128 :], 0.0)
            def bounce_rows(row0):
                w = row0 // HR
                return bounces[w], row0 - w * HR

            bt, r0 = bounce_rows(0)
            nc.sync.dma_start(bt[r0 : r0 + 128, :], h_sb[:, 0:128])
            for gz in range(G + 1, BG):
                bt, r0 = bounce_rows(gz * 128)
                nc.sync.dma_start(bt[r0 : r0 + 128, :], h_sb[:, 0:128])

            for l in range(L):
                if l < L - 1:
                    nc.sync.dma_start(bd_sb[:], bd_in[l, :, :])
                # A: h = xt.T @ W_T per group (node-major), cast to bf16
                for g in range(G):
                    hps = psp.tile([128, 128], DT.float32, name="hps", tag="hps")
                    nc.tensor.matmul(
                        hps[:],
                        lhsT=xt_sb[:, g * 128 : (g + 1) * 128],
                        rhs=wt_sb[:, l * 128 : (l + 1) * 128],
                        start=True, stop=True,
                    )
                    nc.scalar.copy(h_sb[:, (g + 1) * 128 : (g + 2) * 128], hps[:])
                    # B: stream each group's h rows to the bounce buffer
                    bt, r0 = bounce_rows((g + 1) * 128)
                    nc.sync.dma_start(
                        bt[r0 : r0 + 128, :],
                        h_sb[:, (g + 1) * 128 : (g + 2) * 128],
                    )
                # init agg with the self-loop term: agg = dinv*h (local rows)
                nc.vector.tensor_copy(agg_sb[:, : G * 128],
                                      h_sb[:, 128 : (G + 1) * 128])
                # C: AllGather each window half
                if not no_cc:
                    for w in (0, 1):
                        nc.gpsimd.collective_compute(
                            "AllGather", mybir.AluOpType.bypass,
                            replica_groups=[list(range(C))],
                            ins=[bounces[w][:]],
                            outs=[tables[l][w][:]],
                        )
                # D/E: gather + segment reduce
                for bi, b in enumerate(p.batches if not no_gather else []):
                    w = b["w"]
                    nch = b["chunks"]
                    msgs = mp.tile([128, nch * 128], DT.bfloat16,
                                   name="msgs", tag="msgs")
                    nc.gpsimd.dma_gather(
                        out_ap=msgs[:].rearrange("p (c f) -> p c f", f=128),
                        in_ap=tables[l][w][:],
                        idxs_ap=idx_sb[:, b["col"] : b["col"] + b["n"] // 16],
                        num_idxs=b["n"],
                        num_idxs_reg=b["n"],
                        elem_size=128,
                        single_packet=False,
                        queue_num=bi % 4,
                    )
                    for (g0, m, t0, T, choff) in (b["rects"] if not no_reduce else []):
                        W = m * 128
                        base = choff * 128
                        # in-place binary-tree halving over the T chunk-slabs
                        t = T
                        while t > 1:
                            h = t // 2
                            lo = msgs[:, base : base + h * W]
                            hi = msgs[:, base + (t - h) * W : base + t * W]
                            nc.vector.tensor_add(lo, lo, hi)
                            t = t - h
                        aslice = agg_sb[:, g0 * 128 : (g0 + m) * 128]
                        nc.vector.tensor_add(aslice, aslice,
                                             msgs[:, base : base + W])
                # F/G: epilogue
                for g in range(G):
                    gs = slice(g * 128, (g + 1) * 128)
                    if l < L - 1:
                        tps = psp.tile([128, 128], DT.bfloat16, name="tps", tag="tps")
                        nc.tensor.transpose(out=tps[:], in_=agg_sb[:, gs],
                                            identity=ident[:])
                        etmp = wp.tile([128, 128], DT.float32, name="etmp", tag="etmp")
                        nc.vector.tensor_mul(etmp[:], tps[:], dinv2_sb[:, gs])
                        nc.vector.tensor_add(xt_sb[:, gs], etmp[:], bd_sb[:, gs])
                    else:
                        xo = wp.tile([128, 128], DT.float32, name="xo", tag="xo")
                        nc.vector.tensor_scalar_mul(xo[:], agg_sb[:, gs],
                                                    dinvc_sb[:, g : g + 1])
                        nc.vector.tensor_add(xo[:], xo[:], brep_sb[:])
                        nc.sync.dma_start(out_ext[g * 128 : (g + 1) * 128, :], xo[:])
    return nc


def assemble_output(p: Plan, outs):
    """outs: list of per-core 'out' arrays [P,128] -> full [N,128]."""
    full = np.empty((p.N, 128), np.float32)
    full[:] = np.stack([outs[c] for c in range(p.C)], axis=0)[
        p.core_of, p.pos_of
    ]
    return full


def make_in_maps(p: Plan):
    ins = p.inputs
    return [
        dict(
            xt=np.ascontiguousarray(ins["xt"][c]),
            dinv2=np.ascontiguousarray(ins["dinv2"][c]),
            bd=np.ascontiguousarray(ins["bd"][c]),
            dinvc=np.ascontiguousarray(ins["dinvc"][c]),
            brep=np.ascontiguousarray(ins["brep"]),
            wt=np.ascontiguousarray(ins["wt"]),
            idxs=np.ascontiguousarray(ins["idxs"][c]),
        )
        for c in range(p.C)
    ]




def kernel(x, edge_index, lin_w, gcn_w, gcn_b, n_cores=8):
    """Full inputs in, full output out. Shards internally across 8 cores."""
    x = np.asarray(x, np.float32)
    edge_index = np.asarray(edge_index)
    in_dtype = edge_index.dtype
    p = build_plan(x, edge_index, lin_w, gcn_w, gcn_b, C=n_cores)
    nc = build_nc(p)
    nc.finalize()
    in_maps = make_in_maps(p)
    res = run_bass_kernel_spmd(nc, in_maps, core_ids=list(range(n_cores)))
    outs = [res.results[c]["out"] for c in range(n_cores)]
    return assemble_output(p, outs)



# revision 3
# speedup vs baseline: 1.1987x; 1.1987x over previous
"""Trainium2 Bass kernel: 4-layer GCN message passing on 8 NeuronCores.

Self-contained: host-side graph partitioning/padded-CSR planning + Bass/Tile
device program (matmul -> AllGather -> bulk dma_gather -> tree segment-reduce),
executed via run_bass_kernel_spmd on cores 0-7.

Pipelined structure: each layer's gather batches are split by destination half
(LO = window-0 resident dst groups, HI = window-1 residents). When the LO half
finishes reducing, its epilogue + next-layer matmul + bounce + AllGather(w0)
issue immediately and overlap the HI half's gathers, hiding collectives and
epilogues inside the gather stream.

kernel(**inputs) takes the FULL unsharded inputs (x, edge_index, lin_w, gcn_w,
gcn_b) and returns the FULL [N, 128] float32 output.
"""


import math
from dataclasses import dataclass, field

import numpy as np
import ml_dtypes

import concourse.bacc as bacc
import concourse.bass as bass
import concourse.mybir as mybir
import concourse.tile as tile
from concourse.bass_utils import run_bass_kernel_spmd
from concourse.masks import make_identity
from concourse import library_config

BF16 = ml_dtypes.bfloat16
MAXCH = 64    # max msgs chunks (128 slots each) per dma_gather batch


@dataclass
class Plan:
    C: int
    L: int
    N: int
    G: int        # real groups of 128 pi-slots per core
    P: int        # pi-slots per core = G*128
    Nc: int       # real nodes per core
    BG: int       # bounce groups (incl zero group(s))
    BR: int       # bounce rows = BG*128
    HR: int       # rows per window half
    WSZ: int      # table rows per window = C*HR
    GLO: int      # dst groups resident in window 0 (groups [0, GLO))
    COLS: int     # idx stream columns (16 idx each)
    batches: list = field(default_factory=list)
    # host-side tensors (per core where applicable)
    inputs: dict = field(default_factory=dict)
    core_of: np.ndarray = None
    pos_of: np.ndarray = None


def build_plan(x, edge_index, lin_w, gcn_w, gcn_b, C=8):
    N, D = x.shape
    L = lin_w.shape[0]
    assert D == 128 and N % C == 0
    x = np.asarray(x, np.float32)
    src = np.asarray(edge_index[0], np.int64)
    dst = np.asarray(edge_index[1], np.int64)

    deg = (np.bincount(dst, minlength=N) + 1).astype(np.float64)
    dinv = 1.0 / np.sqrt(deg)

    Nc = N // C
    G = (Nc + 127) // 128
    P = G * 128
    BG = G + 1
    if BG % 2:
        BG += 1
    BR = BG * 128
    HR = BR // 2
    while max(HR, 128 + Nc) >= BR:
        BG += 2
        BR = BG * 128
        HR = BR // 2
    WSZ = C * HR
    assert WSZ <= 32768, f"window {WSZ} exceeds int16 index range"
    zrow_w1 = max(HR, 128 + Nc)   # first all-zero bounce row in window 1

    # Greedy per-node window choice: place each src so its dsts' in-edge
    # counts stay balanced across the two windows (cuts padded-CSR K).
    # Global capacities; cores are re-dealt afterwards.
    cap0_full = min(HR - 128, Nc)         # pos < cap0 -> window 0
    GLO = cap0_full // 128
    gcap = np.array([C * cap0_full, C * (Nc - cap0_full)], np.int64)
    aso = np.argsort(src, kind="stable")
    adj_dst = dst[aso]
    a_start = np.searchsorted(src[aso], np.arange(N + 1))
    out_deg = np.diff(a_start)
    sorder = np.argsort(-out_deg, kind="stable")
    cnt01 = np.zeros((2, N), np.int32)
    win_node = np.empty(N, np.int8)
    for n in sorder:
        if gcap[0] <= 0:
            w = 1
        elif gcap[1] <= 0:
            w = 0
        else:
            dl = adj_dst[a_start[n] : a_start[n + 1]]
            s0 = int(cnt01[0, dl].sum())
            s1 = int(cnt01[1, dl].sum())
            if s0 != s1:
                w = 0 if s0 < s1 else 1
            else:
                w = 0 if gcap[0] >= gcap[1] else 1
        win_node[n] = w
        gcap[w] -= 1
        np.add.at(cnt01[w], adj_dst[a_start[n] : a_start[n + 1]], 1)

    # refinement sweeps: reassign each src to the window that best balances
    # its dsts' in-edge counts, via capacity-preserving swaps.
    for _ in range(2):
        gain = np.zeros(N, np.int64)
        want = np.full(N, -1, np.int8)
        for n in sorder:
            dl = adj_dst[a_start[n] : a_start[n + 1]]
            w = win_node[n]
            s_cur = int(cnt01[w, dl].sum()) - len(dl)   # excl own edges
            s_oth = int(cnt01[1 - w, dl].sum())
            if s_oth < s_cur:
                gain[n] = s_cur - s_oth
                want[n] = 1 - w
        movers0 = np.where((want == 1) & (gain > 0))[0]
        movers1 = np.where((want == 0) & (gain > 0))[0]
        movers0 = movers0[np.argsort(-gain[movers0], kind="stable")]
        movers1 = movers1[np.argsort(-gain[movers1], kind="stable")]
        k = min(len(movers0), len(movers1))
        if k == 0:
            break
        for a, b in zip(movers0[:k], movers1[:k]):
            for n in (a, b):
                w = win_node[n]
                dl = adj_dst[a_start[n] : a_start[n + 1]]
                np.add.at(cnt01[w], dl, -1)
                win_node[n] = 1 - w
                np.add.at(cnt01[1 - w], dl, 1)

    # per-node in-edge window counts (K0, K1) incl self loop
    k0 = np.bincount(dst[win_node[src] == 0], minlength=N)
    k1 = np.bincount(dst[win_node[src] == 1], minlength=N)

    # re-deal: within each window class, sort by (K0+K1, K0) desc and deal
    # round-robin so every core's group g has a near-identical K profile
    core_of = np.empty(N, np.int64)
    pos_of = np.empty(N, np.int64)
    for w in (0, 1):
        base = 0 if w == 0 else cap0_full
        members = np.where(win_node == w)[0]
        key = (k0[members] + k1[members]) * 1000 + np.maximum(k0[members],
                                                              k1[members])
        members = members[np.argsort(-key, kind="stable")]
        core_of[members] = np.arange(len(members)) % C
        pos_of[members] = base + np.arange(len(members)) // C

    rows = 128 + pos_of
    win_of_node = rows // HR                      # 0 or 1
    rel_of_node = core_of * HR + (rows % HR)      # window-relative table slot
    assert (win_of_node == win_node).all()

    # edges keyed by (dst core, dst pos, src window). Self-loops are NOT
    # gathered: the device initializes agg = dinv^2-scaled local h instead.
    es = src
    ed = dst
    kc = core_of[ed]
    kp = pos_of[ed]
    kw = win_of_node[es]
    key = (kc * P + kp) * 2 + kw
    so = np.argsort(key, kind="stable")
    key_s = key[so]
    val_s = rel_of_node[es[so]]
    cnt = np.bincount(key_s, minlength=C * P * 2)
    starts = np.concatenate([[0], np.cumsum(cnt)])[:-1]
    rank_in = np.arange(len(key_s)) - starts[key_s]

    # per-group per-window K: cross-core max (groups are K-profile matched
    # across cores by the re-deal, so this is tight)
    Kg = cnt.reshape(C, G, 128, 2).max(axis=(0, 2))   # [G, 2]
    Kmax = max(int(Kg.max()), 1)

    z0 = 0                   # row 0 of rank 0 block (zero group)
    z1 = zrow_w1 - HR        # rank 0 block, window 1

    idxtab = np.empty((C * P * 2, Kmax), np.int16)
    idxtab.reshape(C * P, 2, Kmax)[:, 0, :] = z0
    idxtab.reshape(C * P, 2, Kmax)[:, 1, :] = z1
    assert val_s.max() < 32768
    assert rank_in.max() < Kmax or Kmax >= cnt.max()
    idxtab[key_s, rank_in] = val_s.astype(np.int16)

    # Rectangles: decompose each (dst half, window) active staircase into
    # maximal (g0, m) x (t0, T) rectangles. Each rectangle is reduced with an
    # in-place binary tree over its T chunk-slabs, then one add into agg.
    def extract_rects(Kw, g0, m, t0):
        if m == 0:
            return []
        kmin = int(Kw[g0 : g0 + m].min())
        out = []
        if kmin > t0:
            out.append((g0, m, t0, kmin - t0))
        # split at groups with Kg == kmin, recurse on sub-runs above kmin
        g = g0
        while g < g0 + m:
            if Kw[g] == kmin:
                g += 1
                continue
            g1 = g
            while g1 < g0 + m and Kw[g1] > kmin:
                g1 += 1
            out.extend(extract_rects(Kw, g, g1 - g, kmin))
            g = g1
        return out

    # phases: (dst half, src window) in order (LO,0) (LO,1) (HI,0) (HI,1)
    phase_ranges = [(0, GLO, 0), (0, GLO, 1), (GLO, G, 0), (GLO, G, 1)]
    batches = []
    for pi, (glo, ghi, w) in enumerate(phase_ranges):
        Kw = Kg[:, w]
        rects = []
        g = glo
        while g < ghi:
            if Kw[g] == 0:
                g += 1
                continue
            g1 = g
            while g1 < ghi and Kw[g1] > 0:
                g1 += 1
            for (rg0, rm, rt0, rT) in extract_rects(Kw, g, g1 - g, 0):
                rects.append((rg0, rm, rt0, rT))
            g = g1
        # split rectangles that exceed MAXCH chunks (along t), then pack
        split = []
        for (g0, m, t0, T) in rects:
            assert m <= MAXCH, f"group run m={m} exceeds MAXCH={MAXCH}"
            tcap = max(1, MAXCH // m)
            t = t0
            while t < t0 + T:
                tt = min(tcap, t0 + T - t)
                split.append((g0, m, t, tt))
                t += tt
        cur = None
        for (g0, m, t0, T) in split:
            if cur is None or cur["chunks"] + m * T > MAXCH:
                cur = {"w": w, "phase": pi, "chunks": 0, "rects": []}
                batches.append(cur)
            cur["rects"].append((g0, m, t0, T, cur["chunks"]))
            cur["chunks"] += m * T
    off = 0
    for b in batches:
        b["n"] = b["chunks"] * 128
        b["col"] = off
        off += b["n"] // 16
    COLS = off

    # per-core idx streams: per rect, [T, m, 128] slab (t-major chunks)
    tabv = idxtab.reshape(C, P, 2, Kmax)
    streams = np.empty((C, 16, COLS), np.int16)
    for c in range(C):
        parts = []
        for b in batches:
            for (g0, m, t0, T, _) in b["rects"]:
                slab = tabv[c, g0 * 128 : (g0 + m) * 128, b["w"], t0 : t0 + T]
                parts.append(np.ascontiguousarray(slab.T).ravel())  # [T, m*128]
        s = np.concatenate(parts)
        streams[c] = s.reshape(-1, 16).T
    idxs_in = np.tile(streams, (1, 8, 1))   # replicate over 8 q7 cores -> [C,128,COLS]

    # dense per-core inputs
    nid = np.full((C, P), 0, np.int64)
    vmask = np.zeros((C, P), bool)
    nid[core_of, pos_of] = np.arange(N)
    vmask[core_of, pos_of] = True
    dinv_loc = np.where(vmask, dinv[nid], 0.0)                       # [C,P] f64
    xloc = np.where(vmask[..., None], x[nid], 0.0) * dinv_loc[..., None]
    xt_in = np.ascontiguousarray(xloc.transpose(0, 2, 1)).astype(BF16)  # [C,128,P]
    dinv2_in = np.broadcast_to(
        (dinv_loc**2).astype(np.float32)[:, None, :], (C, 128, P)
    ).copy()
    bias = np.asarray(gcn_b, np.float64)                             # [L,128]
    bd_in = (bias[None, :, :, None] * dinv_loc[:, None, None, :]).astype(BF16)  # [C,L,128,P]
    dinvc_in = np.ascontiguousarray(
        dinv_loc.reshape(C, G, 128).transpose(0, 2, 1)
    ).astype(np.float32)                                             # [C,128,G]
    brep_in = np.broadcast_to(
        bias[L - 1].astype(np.float32)[None, :], (128, 128)
    ).copy()                                                         # [128,128]
    W = np.einsum("lij,ljk->lik", np.asarray(gcn_w, np.float64),
                  np.asarray(lin_w, np.float64))                     # [L,128,128]
    wt_in = np.ascontiguousarray(W.transpose(0, 2, 1)).astype(BF16)  # [L,128,128]

    p = Plan(C=C, L=L, N=N, G=G, P=P, Nc=Nc, BG=BG, BR=BR, HR=HR, WSZ=WSZ,
             GLO=GLO, COLS=COLS, batches=batches, core_of=core_of,
             pos_of=pos_of)
    p.inputs = dict(xt=xt_in, dinv2=dinv2_in, bd=bd_in, dinvc=dinvc_in,
                    brep=brep_in, wt=wt_in, idxs=idxs_in)
    return p


def build_nc(p: Plan, debug_mode=""):
    no_gather = "nogather" in debug_mode
    no_reduce = "noreduce" in debug_mode
    no_cc = "nocc" in debug_mode
    C, L, G, P, BG, BR, HR, WSZ, COLS, GLO = (
        p.C, p.L, p.G, p.P, p.BG, p.BR, p.HR, p.WSZ, p.COLS, p.GLO)
    DT = mybir.dt
    nc = bacc.Bacc(None, target_bir_lowering=False, num_swdge_queues=4)

    xt_in = nc.dram_tensor("xt", [128, P], DT.bfloat16, kind="ExternalInput")
    dinv2_in = nc.dram_tensor("dinv2", [128, P], DT.float32, kind="ExternalInput")
    bd_in = nc.dram_tensor("bd", [L, 128, P], DT.bfloat16, kind="ExternalInput")
    dinvc_in = nc.dram_tensor("dinvc", [128, G], DT.float32, kind="ExternalInput")
    brep_in = nc.dram_tensor("brep", [128, 128], DT.float32, kind="ExternalInput")
    wt_in = nc.dram_tensor("wt", [L, 128, 128], DT.bfloat16, kind="ExternalInput")
    idx_in = nc.dram_tensor("idxs", [128, COLS], DT.int16, kind="ExternalInput")
    out_ext = nc.dram_tensor("out", [P, 128], DT.float32, kind="ExternalOutput")

    with tile.TileContext(nc) as tc:
        with (
            tc.tile_pool(name="persist", bufs=1) as pp,
            tc.tile_pool(name="dram", bufs=1, space="DRAM") as dp,
            tc.tile_pool(name="msgs", bufs=5) as mp,
            tc.tile_pool(name="work", bufs=4) as wp,
            tc.tile_pool(name="psum", bufs=4, space="PSUM") as psp,
        ):
            bounces = [dp.tile([HR, 128], DT.bfloat16, name=f"bounce{w}")
                       for w in (0, 1)]
            tables = [
                [
                    dp.tile([WSZ, 128], DT.bfloat16, name=f"tab{l}_{w}",
                            addr_space="Shared")
                    for w in (0, 1)
                ]
                for l in range(L)
            ]

            xt_sb = pp.tile([128, P], DT.bfloat16, name="xt_sb")
            agg_sb = pp.tile([128, P], DT.bfloat16, name="agg_sb")
            h_sb = pp.tile([128, BG * 128], DT.bfloat16, name="h_sb")
            dinv2_sb = pp.tile([128, P], DT.float32, name="dinv2_sb")
            bd_sb = pp.tile([128, P], DT.bfloat16, name="bd_sb")
            dinvc_sb = pp.tile([128, G], DT.float32, name="dinvc_sb")
            brep_sb = pp.tile([128, 128], DT.float32, name="brep_sb")
            wt_sb = pp.tile([128, L * 128], DT.bfloat16, name="wt_sb")
            idx_sb = pp.tile([128, COLS], DT.int16, name="idx_sb")
            ident = pp.tile([128, 128], DT.bfloat16, name="ident")

            nc.sync.dma_start(xt_sb[:], xt_in[:])
            nc.sync.dma_start(dinv2_sb[:], dinv2_in[:])
            nc.sync.dma_start(dinvc_sb[:], dinvc_in[:])
            nc.sync.dma_start(brep_sb[:], brep_in[:])
            nc.sync.dma_start(idx_sb[:], idx_in[:])
            for l in range(L):
                nc.sync.dma_start(wt_sb[:, l * 128 : (l + 1) * 128],
                                  wt_in[l, :, :])
            make_identity(nc, ident[:])
            if no_gather or no_reduce:
                nc.vector.memset(agg_sb[:], 0.0)
            # GPSIMD library switches (standard <-> mlp for DMAGatherAnt) are
            # auto-inserted by Bacc.compile's insert_library_loads pass.

            # zero group 0 (+ any trailing zero groups) of h, mirror into bounce
            nc.vector.memset(h_sb[:, :128], 0.0)
            if BG > G + 1:
                nc.vector.memset(h_sb[:, (G + 1) * 128 :], 0.0)
            def bounce_rows(row0):
                w = row0 // HR
                return bounces[w], row0 - w * HR

            bt, r0 = bounce_rows(0)
            nc.sync.dma_start(bt[r0 : r0 + 128, :], h_sb[:, 0:128])
            for gz in range(G + 1, BG):
                bt, r0 = bounce_rows(gz * 128)
                nc.sync.dma_start(bt[r0 : r0 + 128, :], h_sb[:, 0:128])

            halves = [(0, GLO), (GLO, G)]   # LO (window-0 dst), HI

            def matmul_bounce(l, glo, ghi):
                # h = xt.T @ W_T per group (node-major), stream to bounce,
                # and init agg with the self-loop term agg = h (pre-scaled).
                for g in range(glo, ghi):
                    hps = psp.tile([128, 128], DT.float32, name="hps", tag="hps")
                    nc.tensor.matmul(
                        hps[:],
                        lhsT=xt_sb[:, g * 128 : (g + 1) * 128],
                        rhs=wt_sb[:, l * 128 : (l + 1) * 128],
                        start=True, stop=True,
                    )
                    nc.scalar.copy(h_sb[:, (g + 1) * 128 : (g + 2) * 128], hps[:])
                    bt, r0 = bounce_rows((g + 1) * 128)
                    nc.sync.dma_start(
                        bt[r0 : r0 + 128, :],
                        h_sb[:, (g + 1) * 128 : (g + 2) * 128],
                    )
                nc.vector.tensor_copy(
                    agg_sb[:, glo * 128 : ghi * 128],
                    h_sb[:, (glo + 1) * 128 : (ghi + 1) * 128])

            def all_gather(l, w):
                if no_cc:
                    return
                nc.gpsimd.collective_compute(
                    "AllGather", mybir.AluOpType.bypass,
                    replica_groups=[list(range(C))],
                    ins=[bounces[w][:]],
                    outs=[tables[l][w][:]],
                )

            def gather_phases(l, phases):
                for bi, b in enumerate(p.batches if not no_gather else []):
                    if b["phase"] not in phases:
                        continue
                    w = b["w"]
                    nch = b["chunks"]
                    msgs = mp.tile([128, nch * 128], DT.bfloat16,
                                   name="msgs", tag="msgs")
                    nc.gpsimd.dma_gather(
                        out_ap=msgs[:].rearrange("p (c f) -> p c f", f=128),
                        in_ap=tables[l][w][:],
                        idxs_ap=idx_sb[:, b["col"] : b["col"] + b["n"] // 16],
                        num_idxs=b["n"],
                        num_idxs_reg=b["n"],
                        elem_size=128,
                        single_packet=False,
                        queue_num=bi % 4,
                    )
                    for (g0, m, t0, T, choff) in (b["rects"] if not no_reduce else []):
                        W = m * 128
                        base = choff * 128
                        # in-place binary-tree halving over the T chunk-slabs
                        t = T
                        while t > 1:
                            h = t // 2
                            lo = msgs[:, base : base + h * W]
                            hi = msgs[:, base + (t - h) * W : base + t * W]
                            nc.vector.tensor_add(lo, lo, hi)
                            t = t - h
                        aslice = agg_sb[:, g0 * 128 : (g0 + m) * 128]
                        nc.vector.tensor_add(aslice, aslice,
                                             msgs[:, base : base + W])

            def epilogue(l, glo, ghi):
                for g in range(glo, ghi):
                    gs = slice(g * 128, (g + 1) * 128)
                    if l < L - 1:
                        tps = psp.tile([128, 128], DT.bfloat16, name="tps", tag="tps")
                        nc.tensor.transpose(out=tps[:], in_=agg_sb[:, gs],
                                            identity=ident[:])
                        etmp = wp.tile([128, 128], DT.float32, name="etmp", tag="etmp")
                        nc.vector.tensor_mul(etmp[:], tps[:], dinv2_sb[:, gs])
                        nc.vector.tensor_add(xt_sb[:, gs], etmp[:], bd_sb[:, gs])
                    else:
                        xo = wp.tile([128, 128], DT.float32, name="xo", tag="xo")
                        nc.vector.tensor_scalar_mul(xo[:], agg_sb[:, gs],
                                                    dinvc_sb[:, g : g + 1])
                        nc.vector.tensor_add(xo[:], xo[:], brep_sb[:])
                        nc.sync.dma_start(out_ext[g * 128 : (g + 1) * 128, :], xo[:])

            # ---- layer 0 prologue: full matmul + both bounces + both AGs
            nc.sync.dma_start(bd_sb[:], bd_in[0, :, :])
            matmul_bounce(0, 0, G)
            all_gather(0, 0)
            all_gather(0, 1)

            for l in range(L):
                # LO dst half: gathers from both windows, then its epilogue,
                # next-layer matmul+bounce(w0)+AG(w0) overlap HI gathers.
                gather_phases(l, (0, 1))
                epilogue(l, 0, GLO)
                if l < L - 1:
                    matmul_bounce(l + 1, 0, GLO)
                    all_gather(l + 1, 0)
                gather_phases(l, (2, 3))
                epilogue(l, GLO, G)
                if l < L - 1:
                    nc.sync.dma_start(bd_sb[:], bd_in[l + 1, :, :])
                    matmul_bounce(l + 1, GLO, G)
                    all_gather(l + 1, 1)
    return nc


def assemble_output(p: Plan, outs):
    """outs: list of per-core 'out' arrays [P,128] -> full [N,128]."""
    full = np.empty((p.N, 128), np.float32)
    full[:] = np.stack([outs[c] for c in range(p.C)], axis=0)[
        p.core_of, p.pos_of
    ]
    return full


def make_in_maps(p: Plan):
    ins = p.inputs
    return [
        dict(
            xt=np.ascontiguousarray(ins["xt"][c]),
            dinv2=np.ascontiguousarray(ins["dinv2"][c]),
            bd=np.ascontiguousarray(ins["bd"][c]),
            dinvc=np.ascontiguousarray(ins["dinvc"][c]),
            brep=np.ascontiguousarray(ins["brep"]),
            wt=np.ascontiguousarray(ins["wt"]),
            idxs=np.ascontiguousarray(ins["idxs"][c]),
        )
        for c in range(p.C)
    ]




def kernel(x, edge_index, lin_w, gcn_w, gcn_b, n_cores=8):
    """Full inputs in, full output out. Shards internally across 8 cores."""
    x = np.asarray(x, np.float32)
    edge_index = np.asarray(edge_index)
    p = build_plan(x, edge_index, lin_w, gcn_w, gcn_b, C=n_cores)
    nc = build_nc(p)
    nc.finalize()
    in_maps = make_in_maps(p)
    res = run_bass_kernel_spmd(nc, in_maps, core_ids=list(range(n_cores)))
    outs = [res.results[c]["out"] for c in range(n_cores)]
    return assemble_output(p, outs)
